# revision 21
# baseline (speedup 1.0000x reference)
"""GCN + LSTM kernel for Trainium2, 8-core SPMD.

Reference semantics:
  1. GCN layer with symmetric normalization over a block-diagonal graph
     (200 graphs x 500 nodes, 1.6M edges), ReLU.
  2. Per-graph mean pooling -> [200, 128].
  3. Sliding windows (len 20) -> single-layer LSTM -> FC -> [181, 1].

Sharding: graph/data parallel. Core c owns graphs [25c, 25c+25) == nodes
[12500c, 12500(c+1)); edges never cross shards because dst lives in src's
graph. Per-graph pooled embeddings are AllGather'd; the LSTM is replicated
on every core.

Device algorithm (no per-edge work on device at all): the host folds the
symmetric normalization into the graph tensors -- x~ = x * outdeg^-1/2
(fp16) and A~[s,d] = count(s->d) * indeg^-1/2[d] (fp8e4, counts are small
ints so the only rounding is the indeg scale; the error averages out over
the 500-node mean pool). The device then computes, per graph,
  agg^T[64,500] = sum_a x~_chunk[128,64]^T @ A~_chunk[128,500]   (PE)
  h3[128,500]   = relu(w_gcn^T @ agg^T + b)                      (PE+ACT)
  pooled[:,g]   = mean_d h3                                      (DVE)
streaming A~ from HBM (memory-regime roofline), then AllGathers the fp16
pooled embeddings [128, 200] and runs the LSTM with hidden on partitions
and the 181 windows on the free dim. Gate pre-activations accumulate
x-proj + h-proj + bias directly in PSUM (3 matmuls per gate, no DVE adds);
gate/state tensors are fp16 for 2x DVE throughput.
"""

import numpy as np

# ---------------------------------------------------------------- constants
N_GRAPHS = 200
NPG = 500  # nodes per graph
N_NODES = N_GRAPHS * NPG
DIN = 64
DGCN = 128
SEQ = 20
H = 128
B_WIN = N_GRAPHS - SEQ + 1  # 181

N_CORES = 8
GPC = N_GRAPHS // N_CORES  # graphs per core: 25
NPC = GPC * NPG  # nodes per core: 12500
P = 128
NSW = 4  # node windows per graph (128 wide; last has 116 rows)
NSLOT = GPC * NSW  # 100 (node = g*500 + 128*a + p, slot s = g*4 + a)


def _cfg_full():
    return dict(n_cores=N_CORES, gpc=GPC, seq=SEQ)


# ---------------------------------------------------------------- device IR
def build_nc(cfg, sim_single=False):
    """sim_single=True builds a 1-device variant with the AllGather replaced
    by local DMA copies (for the TimelineSim offline profiler only)."""
    import concourse.bacc as bacc
    import concourse.tile as tile
    import concourse.mybir as mybir

    f32 = mybir.dt.float32
    f16 = mybir.dt.float16
    f8 = mybir.dt.float8e4
    ACT = mybir.ActivationFunctionType
    AX = mybir.AxisListType

    gpc, seq, n_cores = cfg["gpc"], cfg["seq"], cfg["n_cores"]
    n_graphs_tot = gpc * n_cores
    b_win = n_graphs_tot - seq + 1

    nc = bacc.Bacc(
        "TRN2",
        target_bir_lowering=False,
        debug=False,
        num_devices=1 if sim_single else n_cores,
    )

    # inputs
    xt_in = nc.dram_tensor("xt", [P * NSLOT, DIN], f16, kind="ExternalInput").ap()
    adj_in = nc.dram_tensor("adj", [P, NSLOT * NPG], f8, kind="ExternalInput").ap()
    w_gcn_in = nc.dram_tensor("w_gcn16", [DIN, DGCN], f16, kind="ExternalInput").ap()
    b_gcn_in = nc.dram_tensor("b_gcn", [DGCN, 1], f32, kind="ExternalInput").ap()
    w_ihT_in = nc.dram_tensor("w_ihT16", [DGCN, 4 * H], f16, kind="ExternalInput").ap()
    w_hhT_in = nc.dram_tensor("w_hhT16", [H, 4 * H], f16, kind="ExternalInput").ap()
    b_comb_in = nc.dram_tensor("b_comb16", [1, 4 * H], f16, kind="ExternalInput").ap()
    w_fcT_in = nc.dram_tensor("w_fcT16", [H, 1], f16, kind="ExternalInput").ap()
    b_fc_in = nc.dram_tensor("b_fc", [1, 1], f32, kind="ExternalInput").ap()
    pred_out = nc.dram_tensor("pred", [1, b_win], f32, kind="ExternalOutput").ap()

    with tile.TileContext(nc) as tc:
        with (
            tc.tile_pool(name="dram", bufs=1, space="DRAM") as dpool,
            tc.tile_pool(name="const", bufs=1) as cpool,
            tc.tile_pool(name="work", bufs=3) as wpool,
            tc.tile_pool(name="pagg", bufs=2, space="PSUM") as pagg,
            tc.tile_pool(name="pmm", bufs=2, space="PSUM") as pmm,
            tc.tile_pool(name="pg", bufs=2, space="PSUM") as pg,
        ):
            # ---------------- load inputs
            # xt/adj split into 5 graph-aligned chunk tiles (5 graphs = 20
            # slots each) with interleaved DMA issue, so graph 0's compute
            # starts after the first pair of transfers instead of all six.
            NCH = 5
            SPC = NSLOT // NCH  # slots per chunk: 20
            xt_ch, a_ch = [], []
            xt_r = xt_in[:].rearrange("(p s) d -> p s d", p=P)
            adj_r = adj_in[:].rearrange("p (s d) -> p s d", d=NPG)
            # spread the big input DMAs over all four DGE-capable engines so
            # the transfers run on different queues concurrently
            # weights first (tiny transfers; they gate the per-graph
            # projection pipeline and must not queue behind the big chunks)
            w_gcn_sb = cpool.tile([DIN, DGCN], f16)
            nc.sync.dma_start(w_gcn_sb[:], w_gcn_in[:])
            b_gcn_sb = cpool.tile([DGCN, 1], f32)
            nc.sync.dma_start(b_gcn_sb[:], b_gcn_in[:])
            w_ihT_sb = cpool.tile([DGCN, 4 * H], f16)
            nc.sync.dma_start(w_ihT_sb[:], w_ihT_in[:])
            w_hhT_sb = cpool.tile([H, 4 * H], f16)
            nc.sync.dma_start(w_hhT_sb[:], w_hhT_in[:])
            b_comb_sb = cpool.tile([1, 4 * H], f16)
            nc.sync.dma_start(b_comb_sb[:], b_comb_in[:])
            w_fcT_sb = cpool.tile([H, 1], f16)
            nc.sync.dma_start(w_fcT_sb[:], w_fcT_in[:])
            b_fc_sb = cpool.tile([1, 1], f32)
            nc.sync.dma_start(b_fc_sb[:], b_fc_in[:])
            qs = [nc.sync, nc.gpsimd]
            for j in range(NCH):
                xt_j = cpool.tile([P, SPC, DIN], f16, tag=f"xt{j}")
                qs[j % 2].dma_start(xt_j[:], xt_r[:, j * SPC : (j + 1) * SPC, :])
                a_j = cpool.tile([P, SPC, NPG], f8, tag=f"adj{j}")
                qs[(j + 1) % 2].dma_start(
                    a_j[:], adj_r[:, j * SPC : (j + 1) * SPC, :]
                )
                xt_ch.append(xt_j)
                a_ch.append(a_j)

            ones_row = cpool.tile([1, b_win], f16)
            nc.vector.memset(ones_row[:], 1.0)

            pooled32 = cpool.tile([P, gpc], f32)

            # ---------------- GCN: per-graph dense SpMM + project + pool
            # The projection matmul for graph g-1 is emitted after graph g's
            # aggregation matmuls so the PE never head-of-line stalls waiting
            # for the PSUM->SBUF copy.
            pending = None  # (g, aggs tile) awaiting projection

            def emit_proj(g, aggs):
                h3p = pmm.tile([DGCN, NPG], f32, tag="mm")
                nc.tensor.matmul(h3p[:], w_gcn_sb[:], aggs[:], start=True, stop=True)
                h3 = wpool.tile([DGCN, NPG], f16, tag="h3")
                nc.scalar.activation(h3[:], h3p[:], ACT.Relu, bias=b_gcn_sb[:])
                nc.vector.reduce_sum(pooled32[:, g : g + 1], h3[:], AX.X)

            GPCH = gpc // NCH  # graphs per chunk: 5
            for g in range(gpc):
                xt_j = xt_ch[g // GPCH]
                a_j = a_ch[g // GPCH]
                s0 = (g % GPCH) * NSW
                aggp = pagg.tile([DIN, NPG], f32, tag="agg")
                for a in range(NSW):
                    nc.tensor.matmul(
                        aggp[:],
                        xt_j[:, s0 + a, :],
                        a_j[:, s0 + a, :],
                        start=(a == 0),
                        stop=(a == NSW - 1),
                    )
                aggs = wpool.tile([DIN, NPG], f16, tag="aggs")
                if g % 2 == 0:
                    nc.scalar.copy(aggs[:], aggp[:])
                else:
                    nc.vector.tensor_copy(aggs[:], aggp[:])
                if pending is not None:
                    emit_proj(*pending)
                pending = (g, aggs)
            emit_proj(*pending)

            pooled16 = cpool.tile([P, gpc], f16)
            nc.scalar.mul(pooled16[:], pooled32[:], 1.0 / NPG)

            # ---------------- all-gather pooled embeddings (fp16)
            cc_in = dpool.tile([P, gpc], f16)
            cc_out = dpool.tile([P * n_cores, gpc], f16)
            nc.sync.dma_start(cc_in[:], pooled16[:])
            if sim_single:
                for c in range(n_cores):
                    nc.sync.dma_start(cc_out[c * P : (c + 1) * P, :], cc_in[:])
            else:
                nc.gpsimd.collective_compute(
                    "AllGather",
                    mybir.AluOpType.bypass,
                    replica_groups=[list(range(n_cores))],
                    ins=[cc_in.opt()],
                    outs=[cc_out.opt()],
                )
            hgT16 = cpool.tile([P, n_graphs_tot], f16)
            nc.sync.dma_start(
                hgT16[:].rearrange("p (c g) -> p c g", c=n_cores),
                cc_out[:].rearrange("(c p) g -> p c g", p=P),
            )

            # ---------------- LSTM (windows on free dim, hidden on partitions)
            cT16 = cpool.tile([H, b_win], f16)
            hT16 = cpool.tile([H, b_win], f16)
            nc.vector.memset(cT16[:], 0.0)
            nc.vector.memset(hT16[:], 0.0)

            # gate order (PyTorch): 0=i, 1=f, 2=g, 3=o
            # Gate pre-activations live in PSUM pairs: g01 = (f, i), g23 =
            # (g, o). Each gate's 3 accumulating matmuls (bias outer-product,
            # x-proj, recurrent) are contiguous so only one accumulation
            # group is ever open per PSUM bank. The recurrence's critical
            # cycle is h -> whh_f -> sigma_f -> t1 -> c -> tanh_c -> h, so
            # the f gate's matmuls are emitted first and the (f, i) sigmoid
            # is one batched op.
            def gate_mms(dst, k, l):
                """dst <- b_k + w_ih_k^T x_l + w_hh_k^T h  (PSUM accumulate)."""
                nc.tensor.matmul(
                    dst, b_comb_sb[:, k * H : (k + 1) * H], ones_row[:],
                    start=True, stop=False,
                )
                nc.tensor.matmul(
                    dst, w_ihT_sb[:, k * H : (k + 1) * H],
                    hgT16[:, l : l + b_win],
                    start=False, stop=False,
                )
                nc.tensor.matmul(
                    dst, w_hhT_sb[:, k * H : (k + 1) * H], hT16[:],
                    start=False, stop=True,
                )

            for l in range(seq):
                g01 = pg.tile([H, 2, b_win], f32, tag="g01")  # slots: f, i
                g23 = pg.tile([H, 2, b_win], f32, tag="g23")  # slots: g, o
                gate_mms(g01[:, 0, :], 1, l)  # f first: unblocks sigma
                gate_mms(g01[:, 1, :], 0, l)  # i
                gate_mms(g23[:, 0, :], 2, l)  # g
                gate_mms(g23[:, 1, :], 3, l)  # o

                gif = wpool.tile([H, 2, b_win], f16, tag="gif")
                nc.scalar.activation(gif[:], g01[:], ACT.Sigmoid)
                gg = wpool.tile([H, b_win], f16, tag="gg")
                nc.scalar.activation(gg[:], g23[:, 0, :], ACT.Tanh)
                go = wpool.tile([H, b_win], f16, tag="go")
                nc.scalar.activation(go[:], g23[:, 1, :], ACT.Sigmoid)

                t1 = wpool.tile([H, b_win], f16, tag="t1")
                nc.vector.tensor_mul(t1[:], gif[:, 0, :], cT16[:])
                t2 = wpool.tile([H, b_win], f16, tag="t2")
                nc.vector.tensor_mul(t2[:], gif[:, 1, :], gg[:])
                nc.vector.tensor_add(cT16[:], t1[:], t2[:])
                tch = wpool.tile([H, b_win], f16, tag="tch")
                nc.scalar.activation(tch[:], cT16[:], ACT.Tanh)
                nc.vector.tensor_mul(hT16[:], go[:], tch[:])

            prt = pg.tile([H, 2, b_win], f32, tag="g01")
            nc.tensor.matmul(
                prt[0:1, 0, :], w_fcT_sb[:], hT16[:], start=True, stop=True
            )
            pred_t = wpool.tile([1, b_win], f32, tag="pred")
            nc.scalar.activation(
                pred_t[:], prt[0:1, 0, :], ACT.Identity, bias=b_fc_sb[:]
            )
            nc.sync.dma_start(pred_out[:], pred_t[:])

    nc.compile()
    return nc


# ---------------------------------------------------------------- host prep
def make_in_maps(cfg, x, src, dst, w_gcn, b_gcn, w_ih, w_hh, b_ih, b_hh, w_fc, b_fc):
    import ml_dtypes

    f8 = ml_dtypes.float8_e4m3
    gpc, n_cores = cfg["gpc"], cfg["n_cores"]

    x = np.asarray(x, np.float32)
    src = np.asarray(src).astype(np.int64)
    dst = np.asarray(dst).astype(np.int64)

    out_deg = np.maximum(np.bincount(src, minlength=N_NODES), 1.0)
    in_deg = np.maximum(np.bincount(dst, minlength=N_NODES), 1.0)
    alpha = out_deg.astype(np.float32) ** -0.5  # [N]
    beta = in_deg.astype(np.float32) ** -0.5  # [N]

    xs = x * alpha[:, None]  # fold src-side norm into x

    common = {
        "w_gcn16": np.ascontiguousarray(w_gcn.astype(np.float16)),
        "b_gcn": np.ascontiguousarray(b_gcn.astype(np.float32).reshape(DGCN, 1)),
        "w_ihT16": np.ascontiguousarray(w_ih.T.astype(np.float16)),
        "w_hhT16": np.ascontiguousarray(w_hh.T.astype(np.float16)),
        "b_comb16": np.ascontiguousarray(
            (np.asarray(b_ih) + np.asarray(b_hh)).astype(np.float16).reshape(1, 4 * H)
        ),
        "w_fcT16": np.ascontiguousarray(w_fc.T.astype(np.float16)),
        "b_fc": np.ascontiguousarray(np.asarray(b_fc, np.float32).reshape(1, 1)),
    }

    # edge -> (core, partition p, slot s, dst-local d)
    g_all = src // NPG
    core_all = g_all // gpc
    gloc = g_all % gpc
    sloc = src - g_all * NPG
    dloc = dst - g_all * NPG
    p_all = sloc & 127
    a_all = sloc >> 7
    s_all = gloc * NSW + a_all

    in_maps = []
    for c in range(n_cores):
        m = core_all == c
        idx = (p_all[m] * NSLOT + s_all[m]) * NPG + dloc[m]
        counts = np.bincount(idx, minlength=P * NSLOT * NPG).astype(np.float32)
        counts = counts.reshape(P, gpc, NSW, NPG)
        bet = beta[c * NPC : (c + 1) * NPC].reshape(gpc, NPG)
        adj = counts * bet[None, :, None, :]
        adj8 = adj.reshape(P, NSLOT * NPG).astype(f8)

        xc = xs[c * NPC : (c + 1) * NPC].reshape(gpc, NPG, DIN)
        xp = np.zeros((P, NSLOT, DIN), np.float16)
        for a in range(NSW):
            base = 128 * a
            rows = min(NPG - base, P)
            xp[:rows, a::NSW, :] = xc[:, base : base + rows, :].transpose(1, 0, 2)
        in_maps.append(
            {
                "xt": np.ascontiguousarray(xp.reshape(P * NSLOT, DIN)),
                "adj": np.ascontiguousarray(adj8),
                **common,
            }
        )
    return in_maps


# ---------------------------------------------------------------- entry
_CACHE = {}


def kernel(x, src, dst, graph_ids, w_gcn, b_gcn, w_ih, w_hh, b_ih, b_hh, w_fc, b_fc):
    from concourse import bass_utils

    cfg = _cfg_full()
    in_maps = make_in_maps(
        cfg, x, src, dst,
        np.asarray(w_gcn), np.asarray(b_gcn), np.asarray(w_ih), np.asarray(w_hh),
        np.asarray(b_ih), np.asarray(b_hh), np.asarray(w_fc), np.asarray(b_fc),
    )
    if "nc" not in _CACHE:
        _CACHE["nc"] = build_nc(cfg)
    nc = _CACHE["nc"]
    res = bass_utils.run_bass_kernel_spmd(
        nc, in_maps, core_ids=list(range(cfg["n_cores"]))
    )
    pred = res.results[0]["pred"]  # [1, 181]
    return np.ascontiguousarray(pred.reshape(-1, 1).astype(np.float32))
